# revision 14
# baseline (speedup 1.0000x reference)
"""Trainium2 Bass kernel for nn_DSC_86071144612259.

The reference network collapses to a single linear contraction

    u[b, c] = sum_{d<128} sum_{p} W[d, p, c] * y_rev[b, d, p]

where W [128, P, MC] is assembled exactly (float64, on host) from the
small parameter tensors.  The 270 MB y_rev stream is the real work and
is purely HBM bound (~0.45 MB/us per-core share on the sync HWDGE ring
Q1), so y moves as 1 byte/element.  The PE only eats float dtypes, so
bytes are widened to fp16 on-chip.  Measured facts driving the design
(all from HW traces on this problem):

* DMA-completion semaphores become visible to waiting engines ~2.9 us
  after the queue finishes (in-flight pipeline), so every DMA-gated
  stage pays that once; engine-to-engine semaphores are fast (~0.2 us).
* Direct DVE tensor_copy int8->fp16 runs 1.14 us/chunk ([128, 2048]),
  ACT activation-Copy 1.89 us/chunk; together (1.4 chunk/us) they are
  SLOWER than the arrival rate (1.7 chunk/us) and become the critical
  chain.  Instead, DVE chunks stream as *biased uint8* (y+128) and are
  widened by two dual-op tensor_scalars on uint16 views:
      lo = (v & 0x00FF) | 0x6400 ;  hi = (v >> 8) | 0x6400
  0x6400 | b is exactly fp16(1024 + b), so the fp16 lane holds
  y + 128 + 1024 exactly -- and the constant 1152*sum(W) per output
  channel is removed by the final PSUM->SBUF copy, fused as a
  per-partition tensor_scalar subtract.  The packed ops hit the DVE
  2x/4x mode: 0.67 us/chunk, 1.7x the direct cast.
* The last k-chunk is sent as fp16 directly (host pre-scales by
  1/s_row) so the tail pays DMA-visibility + matmul only, no cast.

The per-row dequant scale (and 2^6 for the W/64 tile, which keeps the
biased PSUM inside fp32 headroom) is applied on the HOST on the tiny
[B, 16] output.  Sharding: pure data parallel over batch across 8
cores (2048 rows each); W is a replicated per-core input.
"""

import numpy as np

B = 16384      # batch
L = 129        # history length of y_rev
P = 32         # observation dim
MC = 16        # control dim (output)
H = 24         # spectral dim
M = 64         # filter length
NCORES = 8
BS = B // NCORES           # 2048 batch rows per core
KD = 128                   # delays with nonzero weight
K = KD * P                 # 4096 contraction length
NKC = K // 128             # 32 k-chunks of 128 partitions
CW = BS                    # SBUF columns per chunk (2048)
NFREE = 512                # matmul moving free dim (one fp32 PSUM bank)
NB = BS // NFREE           # 4 batch chunks per core

NI8 = 30                   # chunks 0..29 stream as bytes (DVE/ACT widen)
WSHIFT = 6                 # W tile is W / 2^WSHIFT; host multiplies back

# byte-chunk group structure on the sync HWDGE ring: fine at the head
# (widening starts ASAP after the ~2.9 us visibility lag), coarse in
# the middle, fine again at the tail (drain granularity).
I8_GROUPS = [[0], [1, 2], [3, 4], [5, 6, 7], [8, 9, 10, 11],
             [12, 13, 14, 15], [16, 17, 18, 19], [20, 21, 22, 23],
             [24, 25, 26], [27, 28], [29]]

# ACT (activation-Copy, ~2.0-2.5 us/chunk incl. per-group visibility
# gating) only helps mid-stream; the head and tail must drain on the
# 0.67 us/chunk DVE packed path or they gate the PE.
ACT_CHUNKS = (3, 7, 11, 14, 17, 20, 24)

CONV_ENGINES = ("vector", "scalar")

_CACHE = {}


def _conv_runs():
    grp = {}
    for gi, chunks in enumerate(I8_GROUPS):
        for ci in chunks:
            grp[ci] = gi
    return [("scalar" if ci in ACT_CHUNKS else "vector", ci, grp[ci])
            for ci in range(NI8)]


def _dve_chunks():
    return sorted(ci for e, ci, g in _conv_runs() if e == "vector")


def _build_w(M0, M_tilde, M_0l, M_big, sigma, lambda_e, phi, phi_tilde):
    """Collapse the parameter tensors into W [KD, MC, P] (float64).

    Mirrors reference.py exactly:
      term1: delay 0,      M0
      term2: delays 1..64, sum_i lambda_i^0.25 phi_tilde[j-1,i] M_tilde[i]
      term3: delays 0..63, sum_l sigma_l^0.25  phi[k,l]         M_0l[l]
      term4: delays 1..127 via conv(phi_tilde[:,i], phi[:,l]) and M_big
    """
    f8 = np.float64
    M0 = M0.astype(f8)
    M_tilde = M_tilde.astype(f8)
    M_0l = M_0l.astype(f8)
    M_big = M_big.astype(f8)
    sigma = sigma.astype(f8)
    lambda_e = lambda_e.astype(f8)
    phi = phi.astype(f8)
    phi_tilde = phi_tilde.astype(f8)

    W = np.zeros((KD, MC, P), dtype=f8)
    W[0] += M0
    pt = phi_tilde * (lambda_e ** 0.25)[None, :]
    W[1:M + 1] += np.einsum("ji,icp->jcp", pt, M_tilde)
    ps = phi * (sigma ** 0.25)[None, :]
    W[0:M] += np.einsum("kl,lcp->kcp", ps, M_0l)
    W4 = np.empty((H, H, 2 * M - 1), dtype=f8)
    for i in range(H):
        for l in range(H):
            W4[i, l] = np.convolve(phi_tilde[:, i], phi[:, l])
    scale = (lambda_e[:, None] * sigma[None, :]) ** 0.25
    W[1:2 * M] += np.einsum("ild,ilcp->dcp", W4 * scale[:, :, None], M_big)
    return W


def _get_nc():
    """Build the per-core Bass program (cached)."""
    if "nc" in _CACHE:
        return _CACHE["nc"]
    import concourse.bass as bass
    import concourse.mybir as mybir

    runs = _conv_runs()
    # per-chunk: (engine, run-ordinal on that engine) for matmul waits
    chunk_wait = {}
    count = {e: 0 for e in CONV_ENGINES}
    for ename, ci, gi in runs:
        count[ename] += 1
        chunk_wait[ci] = (ename, count[ename])
    assert sorted(chunk_wait) == list(range(NI8))

    nc = bass.Bass("TRN2", target_bir_lowering=False, enable_partition_id=False)
    y8 = nc.dram_tensor("y8", [128, NI8 * CW], mybir.dt.int8, kind="ExternalInput")
    yf = nc.dram_tensor("yf", [128, 2 * CW], mybir.dt.float16,
                        kind="ExternalInput")
    # w columns 0..511: swizzled W/64 fp16; columns 512..513: the fp32
    # bias-correction vector (1152*sum_dve(W)) bit-packed as 2 fp16 cols;
    # columns 514..515: its negation (ACT Identity-bias path).
    w = nc.dram_tensor("w", [128, NKC * MC + 4], mybir.dt.float16,
                       kind="ExternalInput")
    ut = nc.dram_tensor("ut", [128, NFREE], mybir.dt.float16, kind="ExternalOutput")

    y8_sb = nc.alloc_sbuf_tensor("y8_sb", [128, NI8 * CW], mybir.dt.int8)
    y_sb = nc.alloc_sbuf_tensor("y_sb", [128, NI8 * CW], mybir.dt.float16)
    yf_sb = nc.alloc_sbuf_tensor("yf_sb", [128, 2 * CW], mybir.dt.float16)
    w_sb = nc.alloc_sbuf_tensor("w_sb", [128, NKC * MC + 4], mybir.dt.float16)
    u_sb = nc.alloc_sbuf_tensor("u_sb", [128, NFREE], mybir.dt.float16)
    warm_sb = nc.alloc_sbuf_tensor("warm_sb", [128, 4], mybir.dt.float16)
    ps = nc.alloc_psum_tensor("ps", [128, NFREE], mybir.dt.float32)

    sem_g = [nc.alloc_semaphore(f"sem_g{g}") for g in range(len(I8_GROUPS))]
    sem_f = nc.alloc_semaphore("sem_f")     # fp16 chunk 31
    sem_w = nc.alloc_semaphore("sem_w")
    sem_cv = {e: nc.alloc_semaphore(f"sem_cv_{e}") for e in CONV_ENGINES}
    pe_done = nc.alloc_semaphore("pe_done")
    out_done = nc.alloc_semaphore("out_done")
    odma = nc.alloc_semaphore("odma")

    corr_ap = w_sb[:, NKC * MC:NKC * MC + 2].bitcast(mybir.dt.float32)
    ncorr_ap = w_sb[:, NKC * MC + 2:NKC * MC + 4].bitcast(mybir.dt.float32)

    def conv_ops(eng, ename):
        lastg = None
        for ename_r, ci, gi in runs:
            if ename_r != ename:
                continue
            if gi != lastg:
                eng.wait_ge(sem_g[gi], 16)
                lastg = gi
            lo, hi = ci * CW, (ci + 1) * CW
            if ename == "scalar":
                eng.copy(
                    out=y_sb[:, lo:hi], in_=y8_sb[:, lo:hi]
                ).then_inc(sem_cv[ename], 1)
            else:
                xv = y8_sb[:, lo:hi].bitcast(mybir.dt.uint16)
                lov = y_sb[:, lo:lo + CW // 2].bitcast(mybir.dt.uint16)
                hiv = y_sb[:, lo + CW // 2:hi].bitcast(mybir.dt.uint16)
                eng.tensor_scalar(
                    out=lov, in0=xv, scalar1=0x00FF, scalar2=0x6400,
                    op0=mybir.AluOpType.bitwise_and,
                    op1=mybir.AluOpType.bitwise_or,
                )
                eng.tensor_scalar(
                    out=hiv, in0=xv, scalar1=8, scalar2=0x6400,
                    op0=mybir.AluOpType.logical_shift_right,
                    op1=mybir.AluOpType.bitwise_or,
                ).then_inc(sem_cv[ename], 1)

    with nc.Block(no_gpsimd_drain=True) as block:

        @block.sync
        def _(sync):
            for g, chunks in enumerate(I8_GROUPS):
                lo, hi = chunks[0] * CW, (chunks[-1] + 1) * CW
                sync.dma_start(
                    out=y8_sb[:, lo:hi], in_=y8[:, lo:hi]
                ).then_inc(sem_g[g], 16)
            sync.dma_start(out=yf_sb[:, :], in_=yf[:, :]).then_inc(sem_f, 16)
            sync.wait_ge(out_done, 1)
            sync.dma_start(
                out=ut[:, :NFREE // 2], in_=u_sb[:, :NFREE // 2]
            ).then_inc(odma, 16)

        @block.scalar
        def _(scalar):
            # W first (tensor engine blocks on it); then a dummy Copy
            # to pull the ~1.3 us activation-table load out of the
            # first cast's critical path (reads garbage, result unused).
            scalar.dma_start(out=w_sb[:, :], in_=w[:, :]).then_inc(sem_w, 16)
            scalar.copy(out=warm_sb[:, :], in_=y8_sb[:, 0:4])
            conv_ops(scalar, "scalar")
            scalar.wait_ge(out_done, 2)
            scalar.dma_start(
                out=ut[:, NFREE // 2:], in_=u_sb[:, NFREE // 2:]
            ).then_inc(odma, 16)

        @block.tensor
        def _(tensor):
            tensor.wait_ge(sem_w, 16)

            for ci in range(NI8):
                e, n = chunk_wait[ci]
                tensor.wait_ge(sem_cv[e], n)
                for bc in range(NB):
                    tensor.matmul(
                        ps[32 * bc:32 * bc + MC, :],
                        w_sb[:, ci * MC:(ci + 1) * MC],
                        y_sb[:, ci * CW + bc * NFREE:ci * CW + (bc + 1) * NFREE],
                        start=(ci == 0),
                        stop=False,
                        tile_position=(0, 32 * bc),
                    )
            # chunks 30,31 fp16-direct; chunk 31 finishes in N=256
            # halves (h0 first across all bc) so the fused
            # subtract-copies and stores can chase.
            tensor.wait_ge(sem_f, 16)
            for bc in range(NB):
                tensor.matmul(
                    ps[32 * bc:32 * bc + MC, :],
                    w_sb[:, NI8 * MC:(NI8 + 1) * MC],
                    yf_sb[:, bc * NFREE:(bc + 1) * NFREE],
                    start=False,
                    stop=False,
                    tile_position=(0, 32 * bc),
                )
            ci = NI8 + 1
            for half in range(2):
                for bc in range(NB):
                    lo = CW + bc * NFREE + half * (NFREE // 2)
                    hi = lo + NFREE // 2
                    plo, phi_ = half * (NFREE // 2), (half + 1) * (NFREE // 2)
                    tensor.matmul(
                        ps[32 * bc:32 * bc + MC, plo:phi_],
                        w_sb[:, ci * MC:(ci + 1) * MC],
                        yf_sb[:, lo:hi],
                        start=False,
                        stop=True,
                        tile_position=(0, 32 * bc),
                    ).then_inc(pe_done, 1)

        @block.vector
        def _(vector):
            conv_ops(vector, "vector")
            # half-0 matmuls are pe_done incs 1..4
            vector.wait_ge(pe_done, 4)
            vector.tensor_scalar(
                out=u_sb[:, :NFREE // 2], in0=ps[:, :NFREE // 2],
                scalar1=corr_ap, scalar2=None,
                op0=mybir.AluOpType.subtract,
            ).then_inc(out_done, 1)
            vector.wait_ge(pe_done, 8)
            vector.tensor_scalar(
                out=u_sb[:, NFREE // 2:], in0=ps[:, NFREE // 2:],
                scalar1=corr_ap, scalar2=None,
                op0=mybir.AluOpType.subtract,
            ).then_inc(out_done, 2)

    _CACHE["nc"] = nc
    return nc


def _ensure_ntff_hook():
    """bass_utils hard-imports antenv.axon_hooks when BASS_TRACE is set;
    this container's trimmed antenv lacks it.  Register a working stub
    built from trn_agent_boot's ctypes NTFF driver (or a None hook,
    which bass_utils degrades gracefully on)."""
    import importlib.util
    import sys
    import types

    if "antenv.axon_hooks" in sys.modules:
        return
    try:
        if importlib.util.find_spec("antenv.axon_hooks") is not None:
            return
    except (ImportError, ValueError):
        pass
    try:
        from trn_agent_boot.trn_boot import _ntff_profile_via_ctypes

        hook = _ntff_profile_via_ctypes("/opt/axon/libaxon_pjrt.so")
    except Exception:
        hook = None
    mod = types.ModuleType("antenv.axon_hooks")
    mod.get_axon_ntff_profile_hook = lambda: hook
    sys.modules["antenv.axon_hooks"] = mod


def kernel(y_rev, M0, M_tilde, M_0l, M_big, sigma, lambda_e, phi, phi_tilde):
    _ensure_ntff_hook()
    from concourse.bass_utils import run_bass_kernel_spmd

    W = _build_w(M0, M_tilde, M_0l, M_big, sigma, lambda_e, phi, phi_tilde)
    # W_flat[k, c] with k = d*P + p, then swizzled so chunk ki sits at
    # columns [ki*MC, (ki+1)*MC) of a [128, NKC*MC] tile; / 2^WSHIFT
    # keeps PSUM magnitudes inside fp16 range for the output tile.
    Wf = W.transpose(0, 2, 1).reshape(K, MC) * (0.5 ** WSHIFT)
    Wsw = Wf.reshape(NKC, 128, MC).transpose(1, 0, 2).reshape(128, NKC * MC)
    Wd = np.ascontiguousarray(Wsw).astype(np.float16)

    # bias correction: DVE chunks carry y+128, widened to 1024+(y+128) =
    # y + 1152, so PSUM holds u' + 1152*sum_{k in DVE chunks} Wd[k, c].
    dve = _dve_chunks()
    Wd64 = Wd.astype(np.float64)           # the values actually multiplied
    corr = np.zeros(128, dtype=np.float64)  # [32*bc + c] layout, same per bc
    csum = np.zeros(MC, dtype=np.float64)
    for ci in dve:
        csum += 1152.0 * Wd64.reshape(128, NKC, MC)[:, ci, :].sum(axis=0)
    for bc in range(NB):
        corr[32 * bc:32 * bc + MC] = csum
    w_ext = np.empty((128, NKC * MC + 4), dtype=np.float16)
    w_ext[:, :NKC * MC] = Wd
    w_ext[:, NKC * MC:NKC * MC + 2] = (
        corr.astype(np.float32).view(np.float16).reshape(128, 2)
    )
    w_ext[:, NKC * MC + 2:] = (
        (-corr).astype(np.float32).view(np.float16).reshape(128, 2)
    )

    dve_set = set(dve)
    KI = NI8 * 128            # byte-streamed contraction prefix
    in_maps = []
    srows = []
    for sh in range(NCORES):
        blk = y_rev[sh * BS:(sh + 1) * BS, :KD, :].reshape(BS, K)  # [b, k]
        srow = (np.abs(blk).max(axis=1) / 127.0).astype(np.float32)  # [BS]
        np.maximum(srow, 1e-30, out=srow)
        srows.append(srow)
        yn = blk / srow[:, None]                 # |yn| <= 127
        q = np.rint(yn[:, :KI])
        np.clip(q, -127, 127, out=q)
        q = q.astype(np.int8)
        # partition-major layout per chunk: tile[p, j] = q[j, ki*128 + p];
        # DVE chunks additionally biased +128 and column-interleaved so the
        # packed unpack writes halves [0:1024]=even input cols, [1024:2048].
        ytp = np.empty((128, NI8 * CW), dtype=np.int8)
        qT = q.T.reshape(NI8, 128, CW)           # [ki, p, j]
        for ci in range(NI8):
            t = qT[ci]
            if ci in dve_set:
                bt = (t.astype(np.int16) + 128).astype(np.uint8)
                it = np.empty((128, CW), dtype=np.uint8)
                it[:, 0::2] = bt[:, :CW // 2]
                it[:, 1::2] = bt[:, CW // 2:]
                ytp[:, ci * CW:(ci + 1) * CW] = it.view(np.int8)
            else:
                ytp[:, ci * CW:(ci + 1) * CW] = t
        # fp16-direct chunks 30,31: [128, 2*CW], chunk-major columns
        yftp = np.ascontiguousarray(
            yn[:, KI:].astype(np.float16).T
            .reshape(2, 128, CW).transpose(1, 0, 2).reshape(128, 2 * CW))
        in_maps.append({"y8": ytp, "yf": yftp, "w": w_ext})

    res = run_bass_kernel_spmd(_get_nc(), in_maps, list(range(NCORES)))
    _CACHE["last_result"] = res

    out = np.empty((B, MC), dtype=np.float32)
    for sh in range(NCORES):
        # ut[32*bc + c, j] = (u^T[c, bc*512 + j] / srow) / 64
        stripes = res.results[sh]["ut"].reshape(NB, 32, NFREE)[:, :MC, :]
        u = stripes.transpose(0, 2, 1).reshape(BS, MC).astype(np.float32)
        out[sh * BS:(sh + 1) * BS, :] = u * (srows[sh] * float(2 ** WSHIFT))[:, None]
    return out
